# revision 2
# baseline (speedup 1.0000x reference)
"""Multi-head attention (sparse_attention nn_Attention) on 8 TRN2 NeuronCores.

Reference computes standard MHA: project q/k/v, S = qh @ kh^T, attn =
softmax(S * scale), x = attn @ vh; returns (x, attn). The relation_feature
branch in the reference is dead code (results deleted), so relation_feature
is never touched.

Sharding: 8 cores = 4 batches x 2 head-groups. Core c handles batch c//2,
heads [4*(c%2), 4*(c%2)+4). Weights are sliced per head-group, activations
per batch. No collectives: every core writes a disjoint slice of the output.

Host passes pre-transposed operands so every matmul contraction dim lands on
SBUF partitions:
  qT/kT/vT  = q[b].T              [C=512, N=256]
  w{q,k,v}T = W[hg*256:+256, :].T [C=512, GC=256]
Core computes (layout: querytok on partitions through softmax):
  qhT/khT [chan, tok] and vh [tok, chan] projections on PE,
  S[h]    [qtok, ktok] = qhT_h.T @ khT_h      (K=64 per head)
  U       = exp(S*scale) on ACT with fused per-row sum (accum_out)
  attn    = U * (1/rowsum)  -> DMA out + PE-transpose -> attnT
  x       = attnT.T @ vh  accumulated per head into [qtok, chan] PSUM.
"""

import numpy as np

B, N, C = 4, 256, 512
H, HS = 8, 64
HG = 2  # head groups (tensor-parallel over heads)
GH = H // HG  # heads per group = 4
GC = GH * HS  # channels per group = 256
SCALE = HS**-0.5
P = 128

_CACHE = {}


def _build_nc(dt_mm_name: str):
    import concourse.bass as bass  # noqa: F401
    import concourse.mybir as mybir
    import concourse.tile as tile
    from concourse import bacc
    from concourse.masks import make_identity

    f32 = mybir.dt.float32
    dt_mm = getattr(mybir.dt, dt_mm_name)

    nc = bacc.Bacc("TRN2", target_bir_lowering=False)

    qT = nc.dram_tensor("qT", [C, N], f32, kind="ExternalInput")
    kT = nc.dram_tensor("kT", [C, N], f32, kind="ExternalInput")
    vT = nc.dram_tensor("vT", [C, N], f32, kind="ExternalInput")
    wqT = nc.dram_tensor("wqT", [C, GC], f32, kind="ExternalInput")
    wkT = nc.dram_tensor("wkT", [C, GC], f32, kind="ExternalInput")
    wvT = nc.dram_tensor("wvT", [C, GC], f32, kind="ExternalInput")
    out_attn = nc.dram_tensor("out_attn", [GH, N, N], f32, kind="ExternalOutput")
    out_x = nc.dram_tensor("out_x", [N, GC], f32, kind="ExternalOutput")

    KC = C // P  # 4 contraction chunks
    TQ = N // P  # 2 query-token chunks
    TK = N // P  # 2 key-token chunks
    CCH = GC // P  # 2 channel chunks per group

    with tile.TileContext(nc) as tc:
        with (
            tc.tile_pool(name="inputs", bufs=1) as inp,
            tc.tile_pool(name="proj", bufs=1) as proj,
            tc.tile_pool(name="work", bufs=2) as work,
            tc.tile_pool(name="small", bufs=4) as small,
            tc.tile_pool(name="psA", bufs=2, space="PSUM") as psA,
            tc.tile_pool(name="psB", bufs=2, space="PSUM") as psB,
            tc.tile_pool(name="psC", bufs=2, space="PSUM") as psC,
        ):
            ident = inp.tile([P, P], dt_mm)
            make_identity(nc, ident)

            # ---- load inputs, partition-chunked: [(kc p) f] -> [p kc f]
            def load(t, fdim):
                sb = inp.tile([P, KC, fdim], f32, tag=f"in_{t.name}")
                nc.sync.dma_start(sb[:], t.rearrange("(kc p) f -> p kc f", p=P))
                return sb

            qT_sb = load(qT, N)
            kT_sb = load(kT, N)
            vT_sb = load(vT, N)
            wqT_sb = load(wqT, GC)
            wkT_sb = load(wkT, GC)
            wvT_sb = load(wvT, GC)

            if dt_mm != f32:
                def cast(sb, fdim, nm):
                    c = inp.tile([P, KC, fdim], dt_mm, tag=f"cast_{nm}")
                    nc.vector.tensor_copy(c[:], sb[:])
                    return c

                qT_mm = cast(qT_sb, N, "q")
                kT_mm = cast(kT_sb, N, "k")
                vT_mm = cast(vT_sb, N, "v")
                wqT_mm = cast(wqT_sb, GC, "wq")
                wkT_mm = cast(wkT_sb, GC, "wk")
                wvT_mm = cast(wvT_sb, GC, "wv")
            else:
                qT_mm, kT_mm, vT_mm = qT_sb, kT_sb, vT_sb
                wqT_mm, wkT_mm, wvT_mm = wqT_sb, wkT_sb, wvT_sb

            # ---- projections
            # qhT/khT: [chan_part, cc, querytok]  (chan on partitions)
            qhT = proj.tile([P, CCH, N], dt_mm)
            khT = proj.tile([P, CCH, N], dt_mm)
            # vh: [keytok_part, tk, chan]
            vh = proj.tile([P, TK, GC], dt_mm)

            for cc in range(CCH):
                ps_q = psA.tile([P, N], f32, tag="proj_ps")
                for kc in range(KC):
                    nc.tensor.matmul(
                        ps_q,
                        lhsT=wqT_mm[:, kc, cc * P : (cc + 1) * P],
                        rhs=qT_mm[:, kc, :],
                        start=(kc == 0),
                        stop=(kc == KC - 1),
                    )
                nc.any.tensor_copy(qhT[:, cc, :], ps_q)
                ps_k = psA.tile([P, N], f32, tag="proj_ps")
                for kc in range(KC):
                    nc.tensor.matmul(
                        ps_k,
                        lhsT=wkT_mm[:, kc, cc * P : (cc + 1) * P],
                        rhs=kT_mm[:, kc, :],
                        start=(kc == 0),
                        stop=(kc == KC - 1),
                    )
                nc.any.tensor_copy(khT[:, cc, :], ps_k)
            for tk in range(TK):
                ps_v = psA.tile([P, GC], f32, tag="proj_ps")
                for kc in range(KC):
                    nc.tensor.matmul(
                        ps_v,
                        lhsT=vT_mm[:, kc, tk * P : (tk + 1) * P],
                        rhs=wvT_mm[:, kc, :],
                        start=(kc == 0),
                        stop=(kc == KC - 1),
                    )
                nc.any.tensor_copy(vh[:, tk, :], ps_v)

            # ---- attention per head
            x_ps = [
                psC.tile([P, GC], f32, tag="x_ps", name=f"x_ps{i}") for i in range(TQ)
            ]
            for h in range(GH):
                cc = h // (P // HS)  # which chan chunk holds this head
                r0 = (h % (P // HS)) * HS  # partition offset within chunk
                attn_sb = work.tile([P, TQ, N], f32, tag="attn")
                attnT_sb = work.tile([P, TK, N], dt_mm, tag="attnT")
                for tq in range(TQ):
                    s_ps = psA.tile([P, N], f32, tag="s_ps")
                    nc.tensor.matmul(
                        s_ps,
                        lhsT=qhT[r0 : r0 + HS, cc, tq * P : (tq + 1) * P],
                        rhs=khT[r0 : r0 + HS, cc, :],
                        start=True,
                        stop=True,
                    )
                    u = work.tile([P, N], f32, tag="u")
                    dsum = small.tile([P, 1], f32, tag="dsum")
                    nc.scalar.activation(
                        u,
                        s_ps,
                        mybir.ActivationFunctionType.Exp,
                        scale=SCALE,
                        accum_out=dsum,
                    )
                    recip = small.tile([P, 1], f32, tag="recip")
                    nc.vector.reciprocal(recip, dsum)
                    nc.vector.tensor_scalar_mul(attn_sb[:, tq, :], u, recip)
                    nc.sync.dma_start(
                        out_attn[h, tq * P : (tq + 1) * P, :], attn_sb[:, tq, :]
                    )
                # transpose attn -> attnT (keytok on partitions)
                for tk in range(TK):
                    t_ps = psB.tile([P, N], dt_mm, tag="t_ps")
                    for tq in range(TQ):
                        nc.tensor.transpose(
                            t_ps[:, tq * P : (tq + 1) * P],
                            attn_sb[:, tq, tk * P : (tk + 1) * P],
                            ident,
                        )
                    nc.any.tensor_copy(attnT_sb[:, tk, :], t_ps)
                # x += attn_h @ vh_h
                for tq in range(TQ):
                    for tk in range(TK):
                        nc.tensor.matmul(
                            x_ps[tq][:, h * HS : (h + 1) * HS],
                            lhsT=attnT_sb[:, tk, tq * P : (tq + 1) * P],
                            rhs=vh[:, tk, h * HS : (h + 1) * HS],
                            start=(tk == 0),
                            stop=(tk == TK - 1),
                        )

            for tq in range(TQ):
                x_sb = work.tile([P, GC], f32, tag="x_sb")
                nc.any.tensor_copy(x_sb[:], x_ps[tq])
                nc.sync.dma_start(out_x[tq * P : (tq + 1) * P, :], x_sb[:])

    nc.compile()
    return nc


def _get_nc(dt_mm_name: str):
    if dt_mm_name not in _CACHE:
        _CACHE[dt_mm_name] = _build_nc(dt_mm_name)
    return _CACHE[dt_mm_name]


def make_in_maps(q, k, v, W_q, W_k, W_v):
    """Shard full inputs into 8 per-core input dicts (host-side transposes)."""
    f32 = np.float32
    in_maps = []
    for c in range(2 * B):
        b, hg = c // HG, c % HG
        sl = slice(hg * GC, (hg + 1) * GC)
        in_maps.append(
            {
                "qT": np.ascontiguousarray(q[b].T, dtype=f32),
                "kT": np.ascontiguousarray(k[b].T, dtype=f32),
                "vT": np.ascontiguousarray(v[b].T, dtype=f32),
                "wqT": np.ascontiguousarray(W_q[sl, :].T, dtype=f32),
                "wkT": np.ascontiguousarray(W_k[sl, :].T, dtype=f32),
                "wvT": np.ascontiguousarray(W_v[sl, :].T, dtype=f32),
            }
        )
    return in_maps


def assemble(results):
    """Gather 8 per-core outputs into full (x, attn)."""
    x = np.empty((B, N, C), dtype=np.float32)
    attn = np.empty((B, H, N, N), dtype=np.float32)
    for c in range(2 * B):
        b, hg = c // HG, c % HG
        attn[b, hg * GH : (hg + 1) * GH] = results[c]["out_attn"]
        x[b, :, hg * GC : (hg + 1) * GC] = results[c]["out_x"]
    return x, attn


DT_MM = "float32"


def kernel(q, k, v, relation_feature=None, W_q=None, W_k=None, W_v=None,
           W_r_conv=None, W_r_qk=None, _trace=False):
    from concourse.bass_utils import run_bass_kernel_spmd

    nc = _get_nc(DT_MM)
    in_maps = make_in_maps(
        np.asarray(q), np.asarray(k), np.asarray(v),
        np.asarray(W_q), np.asarray(W_k), np.asarray(W_v),
    )
    res = run_bass_kernel_spmd(nc, in_maps, core_ids=list(range(2 * B)), trace=_trace)
    x, attn = assemble(res.results)
    if _trace:
        return (x, attn), res
    return (x, attn)


# revision 8
# speedup vs baseline: 1.5188x; 1.5188x over previous
"""Multi-head attention (sparse_attention nn_Attention) on 8 TRN2 NeuronCores.

Reference computes standard MHA: project q/k/v, S = qh @ kh^T, attn =
softmax(S * scale), x = attn @ vh; returns (x, attn). The relation_feature
branch in the reference is dead code (computed then deleted), so
relation_feature is never touched here.

Sharding: 8 cores = 4 batches x 2 head-groups. Core c handles batch c//2,
heads [4*(c%2), 4*(c%2)+4). Weight slices per head-group, activations per
batch. No collectives: every core writes a disjoint output slice.

Host pre-transposes inputs so every matmul contraction lands on SBUF
partitions, packed [128, KC, 256] partition-major so each DMA partition line
is one contiguous chunk:
  qT/kT/vT  <- q[b].T              (C=512 rows -> KC=4 chunks of 128)
  w{q,k,v}T <- W[hg*256:+256, :].T
Per-core compute (querytok on partitions through softmax):
  qhT/khT [chan, tok], vh [tok, chan] projections on PE
  S[h]    [qtok, ktok] = qhT_h.T @ khT_h      (K=64 per head)
  U       = exp(S*scale) on ACT with fused row-sum (accum_out)
  attn    = U * (1/rowsum)  -> packed [qtok, h, ktok] f32 out
  UT      = PE-transpose(U);  xu = UT.T @ vh;  x = xu * (1/rowsum)
Matmul dtype modes: "float32" (4 cyc/row), "bfloat16" (1 cyc/row, bf16 DMA),
"float32r" (f32 storage bitcast to the fast PE path, 1 cyc/row at N>=256).
"""

import numpy as np

B, N, C = 4, 256, 512
H, HS = 8, 64
HG = 2  # head groups (tensor-parallel over heads)
GH = H // HG  # heads per group = 4
GC = GH * HS  # channels per group = 256
SCALE = HS**-0.5
P = 128
KC = C // P  # 4 contraction chunks
TQ = N // P  # 2 query-token chunks
TK = N // P  # 2 key-token chunks
CCH = GC // P  # 2 channel chunks per group

DT_MM = "bfloat16"  # "float32" | "bfloat16" | "float32r"

_CACHE = {}


def _np_in_dtype(dt_mm_name):
    if dt_mm_name == "bfloat16":
        import ml_dtypes

        return ml_dtypes.bfloat16
    return np.float32


def _build_nc(dt_mm_name: str):
    import concourse.bass as bass  # noqa: F401
    import concourse.mybir as mybir
    import concourse.tile as tile
    from concourse import bacc
    from concourse.masks import make_identity

    f32 = mybir.dt.float32
    dt_mm = getattr(mybir.dt, dt_mm_name)
    # dtype of DRAM inputs + SBUF input tiles; f32r is stored as f32 in
    # SBUF/PSUM and bitcast to float32r at each matmul AP
    dt_in = mybir.dt.bfloat16 if dt_mm_name == "bfloat16" else f32
    dt_store = mybir.dt.bfloat16 if dt_mm_name == "bfloat16" else f32

    def mm_cast(ap):
        return ap.bitcast(dt_mm) if dt_mm_name == "float32r" else ap

    nc = bacc.Bacc("TRN2", target_bir_lowering=False)

    qT = nc.dram_tensor("qT", [P, KC, N], dt_in, kind="ExternalInput")
    kT = nc.dram_tensor("kT", [P, KC, N], dt_in, kind="ExternalInput")
    vT = nc.dram_tensor("vT", [P, KC, N], dt_in, kind="ExternalInput")
    wqT = nc.dram_tensor("wqT", [P, KC, GC], dt_in, kind="ExternalInput")
    wkT = nc.dram_tensor("wkT", [P, KC, GC], dt_in, kind="ExternalInput")
    wvT = nc.dram_tensor("wvT", [P, KC, GC], dt_in, kind="ExternalInput")
    # attn packed [querytok, head, keytok] so DMA rows are 4KB; host unpacks
    out_attn = nc.dram_tensor("out_attn", [N, GH, N], f32, kind="ExternalOutput")
    out_x = nc.dram_tensor("out_x", [N, GC], f32, kind="ExternalOutput")

    with tile.TileContext(nc) as tc:
        with (
            tc.tile_pool(name="inputs", bufs=1) as inp,
            tc.tile_pool(name="proj", bufs=1) as proj,
            tc.tile_pool(name="work", bufs=2) as work,
            tc.tile_pool(name="small", bufs=16) as small,
            tc.tile_pool(name="psA", bufs=2, space="PSUM") as psA,
            tc.tile_pool(name="psB", bufs=2, space="PSUM") as psB,
            tc.tile_pool(name="psC", bufs=2, space="PSUM") as psC,
        ):
            ident = inp.tile([P, P], dt_store)
            make_identity(nc, ident)

            def load(t, fdim):
                sb = inp.tile([P, KC, fdim], dt_in, tag=f"in_{t.name}")
                nc.sync.dma_start(sb[:], t[:])
                return sb

            wqT_sb = load(wqT, GC)
            qT_sb = load(qT, N)
            wkT_sb = load(wkT, GC)
            kT_sb = load(kT, N)
            wvT_sb = load(wvT, GC)
            vT_sb = load(vT, N)

            # ---- projections
            qhT = proj.tile([P, CCH, N], dt_store)  # [chan_part, cc, querytok]
            khT = proj.tile([P, CCH, N], dt_store)
            vh = proj.tile([P, TK, GC], dt_store)  # [keytok_part, tk, chan]

            def project(dst_slice, w_sb, x_sb, m_sl, nfree):
                ps_full = psA.tile([P, 512], f32, tag="proj_ps", name="proj_ps")
                ps = ps_full[:, :nfree]
                for kc in range(KC):
                    nc.tensor.matmul(
                        ps,
                        lhsT=mm_cast(w_sb[:, kc, m_sl]),
                        rhs=mm_cast(x_sb[:, kc, :]),
                        start=(kc == 0),
                        stop=(kc == KC - 1),
                    )
                nc.any.tensor_copy(dst_slice, ps)

            for cc in range(CCH):
                sl = slice(cc * P, (cc + 1) * P)
                project(qhT[:, cc, :], wqT_sb, qT_sb, sl, N)
                project(khT[:, cc, :], wkT_sb, kT_sb, sl, N)
            for tk in range(TK):
                sl = slice(tk * P, (tk + 1) * P)
                project(vh[:, tk, :], vT_sb, wvT_sb, sl, GC)

            # ---- attention: S, exp(+rowsum), normalize, write attn
            u_all = proj.tile([P, TQ, GH, N], dt_store)  # exp(S*scale)
            ut_all = proj.tile([P, GH, TK, TQ * P], dt_store)  # U^T
            recips = {}
            for tq in range(TQ):
                attn_f = work.tile([P, GH, N], f32, tag="attn_f")
                for h in range(GH):
                    cc = h // (P // HS)
                    r0 = (h % (P // HS)) * HS
                    s_ps = psA.tile([P, N], f32, tag="s_ps")
                    nc.tensor.matmul(
                        s_ps,
                        lhsT=mm_cast(qhT[r0 : r0 + HS, cc, tq * P : (tq + 1) * P]),
                        rhs=mm_cast(khT[r0 : r0 + HS, cc, :]),
                        start=True,
                        stop=True,
                    )
                    u = u_all[:, tq, h, :]
                    dsum = small.tile([P, 1], f32, tag="dsum")
                    nc.scalar.activation(
                        u,
                        s_ps,
                        mybir.ActivationFunctionType.Exp,
                        scale=SCALE,
                        accum_out=dsum,
                    )
                    rc = small.tile([P, 1], f32, tag="recip", name=f"rc_{tq}_{h}")
                    nc.vector.reciprocal(rc, dsum)
                    recips[tq, h] = rc
                    nc.vector.tensor_scalar_mul(attn_f[:, h, :], u, rc)
                nc.sync.dma_start(out_attn[tq * P : (tq + 1) * P, :, :], attn_f[:])

            # ---- transpose unnormalized U: [qtok, ktok] -> [ktok, qtok]
            for h in range(GH):
                for tk in range(TK):
                    t_ps = psB.tile([P, TQ * P], dt_store, tag="t_ps")
                    for tq in range(TQ):
                        nc.tensor.transpose(
                            mm_cast(t_ps[:, tq * P : (tq + 1) * P]),
                            mm_cast(u_all[:, tq, h, tk * P : (tk + 1) * P]),
                            mm_cast(ident[:, :]),
                        )
                    nc.any.tensor_copy(ut_all[:, h, tk, :], t_ps)

            # ---- x = attn @ vh (unnormalized, then scaled by recip)
            for tq in range(TQ):
                x_ps = psC.tile([P, GC], f32, tag="x_ps")
                for h in range(GH):
                    for tk in range(TK):
                        nc.tensor.matmul(
                            x_ps[:, h * HS : (h + 1) * HS],
                            lhsT=mm_cast(ut_all[:, h, tk, tq * P : (tq + 1) * P]),
                            rhs=mm_cast(vh[:, tk, h * HS : (h + 1) * HS]),
                            start=(tk == 0),
                            stop=(tk == TK - 1),
                        )
                x_sb = work.tile([P, GC], f32, tag="x_sb")
                for h in range(GH):
                    nc.vector.tensor_scalar_mul(
                        x_sb[:, h * HS : (h + 1) * HS],
                        x_ps[:, h * HS : (h + 1) * HS],
                        recips[tq, h],
                    )
                nc.sync.dma_start(out_x[tq * P : (tq + 1) * P, :], x_sb[:])

    nc.compile()
    return nc


def _get_nc(dt_mm_name: str):
    if dt_mm_name not in _CACHE:
        _CACHE[dt_mm_name] = _build_nc(dt_mm_name)
    return _CACHE[dt_mm_name]


def _pack(a, dtype):
    # [512, F] -> [128, 4, F] partition-major contiguous
    F = a.shape[1]
    return np.ascontiguousarray(
        a.reshape(KC, P, F).transpose(1, 0, 2).astype(dtype)
    )


def make_in_maps(q, k, v, W_q, W_k, W_v, dt_mm_name=None):
    """Shard full inputs into 8 per-core input dicts (host-side transposes)."""
    dt = _np_in_dtype(dt_mm_name or DT_MM)
    in_maps = []
    packed_w = {}
    for hg in range(HG):
        sl = slice(hg * GC, (hg + 1) * GC)
        packed_w[hg] = (
            _pack(np.ascontiguousarray(W_q[sl, :].T), dt),
            _pack(np.ascontiguousarray(W_k[sl, :].T), dt),
            _pack(np.ascontiguousarray(W_v[sl, :].T), dt),
        )
    packed_x = {}
    for b in range(B):
        packed_x[b] = (
            _pack(np.ascontiguousarray(q[b].T), dt),
            _pack(np.ascontiguousarray(k[b].T), dt),
            _pack(np.ascontiguousarray(v[b].T), dt),
        )
    for c in range(2 * B):
        b, hg = c // HG, c % HG
        qTp, kTp, vTp = packed_x[b]
        wqTp, wkTp, wvTp = packed_w[hg]
        in_maps.append(
            {"qT": qTp, "kT": kTp, "vT": vTp, "wqT": wqTp, "wkT": wkTp, "wvT": wvTp}
        )
    return in_maps


def assemble(results):
    """Gather 8 per-core outputs into full (x, attn)."""
    x = np.empty((B, N, C), dtype=np.float32)
    attn = np.empty((B, H, N, N), dtype=np.float32)
    for c in range(2 * B):
        b, hg = c // HG, c % HG
        # out_attn [N, GH, N] -> [GH, N, N]
        attn[b, hg * GH : (hg + 1) * GH] = np.asarray(
            results[c]["out_attn"], dtype=np.float32
        ).transpose(1, 0, 2)
        x[b, :, hg * GC : (hg + 1) * GC] = results[c]["out_x"]
    return x, attn


def kernel(q, k, v, relation_feature=None, W_q=None, W_k=None, W_v=None,
           W_r_conv=None, W_r_qk=None, _trace=False):
    from concourse.bass_utils import run_bass_kernel_spmd

    nc = _get_nc(DT_MM)
    in_maps = make_in_maps(
        np.asarray(q), np.asarray(k), np.asarray(v),
        np.asarray(W_q), np.asarray(W_k), np.asarray(W_v),
    )
    res = run_bass_kernel_spmd(nc, in_maps, core_ids=list(range(2 * B)), trace=_trace)
    x, attn = assemble(res.results)
    if _trace:
        return (x, attn), res
    return (x, attn)


# revision 10
# speedup vs baseline: 1.5674x; 1.0320x over previous
"""Multi-head attention (sparse_attention nn_Attention) on 8 TRN2 NeuronCores.

Reference computes standard MHA: project q/k/v, S = qh @ kh^T, attn =
softmax(S * scale), x = attn @ vh; returns (x, attn). The relation_feature
branch in the reference is dead code (computed then deleted), so
relation_feature is never touched here.

Sharding: 8 cores = 4 batches x 2 head-groups. Core c handles batch c//2,
heads [4*(c%2), 4*(c%2)+4). Weight slices per head-group, activations per
batch. No collectives: every core writes a disjoint output slice.

Host pre-transposes inputs so every matmul contraction lands on SBUF
partitions, packed [128, KC, 256] partition-major so each DMA partition line
is one contiguous chunk:
  qT/kT/vT  <- q[b].T              (C=512 rows -> KC=4 chunks of 128)
  w{q,k,v}T <- W[hg*256:+256, :].T
Per-core compute (querytok on partitions through softmax):
  qhT/khT [chan, tok], vh [tok, chan] projections on PE
  S[h]    [qtok, ktok] = qhT_h.T @ khT_h      (K=64 per head)
  U       = exp(S*scale) on ACT with fused row-sum (accum_out)
  attn    = U * (1/rowsum)  -> packed [qtok, h, ktok] f32 out
  UT      = PE-transpose(U);  xu = UT.T @ vh;  x = xu * (1/rowsum)
Matmul dtype modes: "float32" (4 cyc/row), "bfloat16" (1 cyc/row, bf16 DMA),
"float32r" (f32 storage bitcast to the fast PE path, 1 cyc/row at N>=256).
"""

import numpy as np

B, N, C = 4, 256, 512
H, HS = 8, 64
HG = 2  # head groups (tensor-parallel over heads)
GH = H // HG  # heads per group = 4
GC = GH * HS  # channels per group = 256
SCALE = HS**-0.5
P = 128
KC = C // P  # 4 contraction chunks
TQ = N // P  # 2 query-token chunks
TK = N // P  # 2 key-token chunks
CCH = GC // P  # 2 channel chunks per group

DT_MM = "bfloat16"  # "float32" | "bfloat16" | "float32r"

_CACHE = {}


def _np_in_dtype(dt_mm_name):
    if dt_mm_name == "bfloat16":
        import ml_dtypes

        return ml_dtypes.bfloat16
    return np.float32


def _make_slim_tile(tile):
    """TileContext whose exit skips the two all-engine barriers and ~57
    semaphore clears (~8us tail). Keeps the sync-engine drain (with waits on
    the global clock) so output DMAs complete before the NEFF ends. The NEFF
    then assumes clean semaphores at load (single execution per load, which
    is how run_bass_via_pjrt executes it)."""
    from concourse.vector_clock import ScopedClock

    class SlimTile(tile.TileContext):
        def _drain_and_barrier(self, tick_clock, wait_clock):
            drain_inst = self.nc.sync.drain()
            wait_clock.add_sem_waits(
                drain_inst.ins, ScopedClock({None: tick_clock.global_clock})
            )
            popped = self.nc._tile_sem_poison_stack.pop()
            assert popped is self._sem_poison

    return SlimTile


def _build_nc(dt_mm_name: str):
    import concourse.bass as bass  # noqa: F401
    import concourse.mybir as mybir
    import concourse.tile as tile
    from concourse import bacc
    from concourse.masks import make_identity

    f32 = mybir.dt.float32
    dt_mm = getattr(mybir.dt, dt_mm_name)
    # dtype of DRAM inputs + SBUF input tiles; f32r is stored as f32 in
    # SBUF/PSUM and bitcast to float32r at each matmul AP
    dt_in = mybir.dt.bfloat16 if dt_mm_name == "bfloat16" else f32
    dt_store = mybir.dt.bfloat16 if dt_mm_name == "bfloat16" else f32

    def mm_cast(ap):
        return ap.bitcast(dt_mm) if dt_mm_name == "float32r" else ap

    nc = bacc.Bacc("TRN2", target_bir_lowering=False)

    qT = nc.dram_tensor("qT", [P, KC, N], dt_in, kind="ExternalInput")
    kT = nc.dram_tensor("kT", [P, KC, N], dt_in, kind="ExternalInput")
    vT = nc.dram_tensor("vT", [P, KC, N], dt_in, kind="ExternalInput")
    wqT = nc.dram_tensor("wqT", [P, KC, GC], dt_in, kind="ExternalInput")
    wkT = nc.dram_tensor("wkT", [P, KC, GC], dt_in, kind="ExternalInput")
    wvT = nc.dram_tensor("wvT", [P, KC, GC], dt_in, kind="ExternalInput")
    # attn packed [querytok, head, keytok] so DMA rows are 4KB; host unpacks
    out_attn = nc.dram_tensor("out_attn", [N, GH, N], f32, kind="ExternalOutput")
    out_x = nc.dram_tensor("out_x", [N, GC], f32, kind="ExternalOutput")

    SlimTile = _make_slim_tile(tile)
    with SlimTile(nc) as tc:
        with (
            tc.tile_pool(name="inputs", bufs=1) as inp,
            tc.tile_pool(name="proj", bufs=1) as proj,
            tc.tile_pool(name="work", bufs=2) as work,
            tc.tile_pool(name="small", bufs=16) as small,
            tc.tile_pool(name="psA", bufs=2, space="PSUM") as psA,
            tc.tile_pool(name="psB", bufs=2, space="PSUM") as psB,
            tc.tile_pool(name="psC", bufs=2, space="PSUM") as psC,
        ):
            ident = inp.tile([P, P], dt_store)
            make_identity(nc, ident)

            def load(t, fdim):
                sb = inp.tile([P, KC, fdim], dt_in, tag=f"in_{t.name}")
                nc.sync.dma_start(sb[:], t[:])
                return sb

            wqT_sb = load(wqT, GC)
            qT_sb = load(qT, N)
            wkT_sb = load(wkT, GC)
            kT_sb = load(kT, N)
            wvT_sb = load(wvT, GC)
            vT_sb = load(vT, N)

            # ---- projections
            qhT = proj.tile([P, CCH, N], dt_store)  # [chan_part, cc, querytok]
            khT = proj.tile([P, CCH, N], dt_store)
            vh = proj.tile([P, TK, GC], dt_store)  # [keytok_part, tk, chan]

            def project(dst_slice, w_sb, x_sb, m_sl, nfree):
                ps_full = psA.tile([P, 512], f32, tag="proj_ps", name="proj_ps")
                ps = ps_full[:, :nfree]
                for kc in range(KC):
                    nc.tensor.matmul(
                        ps,
                        lhsT=mm_cast(w_sb[:, kc, m_sl]),
                        rhs=mm_cast(x_sb[:, kc, :]),
                        start=(kc == 0),
                        stop=(kc == KC - 1),
                    )
                nc.any.tensor_copy(dst_slice, ps)

            for cc in range(CCH):
                sl = slice(cc * P, (cc + 1) * P)
                project(qhT[:, cc, :], wqT_sb, qT_sb, sl, N)
                project(khT[:, cc, :], wkT_sb, kT_sb, sl, N)
            for tk in range(TK):
                sl = slice(tk * P, (tk + 1) * P)
                project(vh[:, tk, :], vT_sb, wvT_sb, sl, GC)

            # ---- attention: S, exp(+rowsum), normalize, write attn
            u_all = proj.tile([P, TQ, GH, N], dt_store)  # exp(S*scale)
            ut_all = proj.tile([P, GH, TK, TQ * P], dt_store)  # U^T
            recips = {}
            for tq in range(TQ):
                attn_f = work.tile([P, GH, N], f32, tag="attn_f")
                for h in range(GH):
                    cc = h // (P // HS)
                    r0 = (h % (P // HS)) * HS
                    s_ps = psA.tile([P, N], f32, tag="s_ps")
                    nc.tensor.matmul(
                        s_ps,
                        lhsT=mm_cast(qhT[r0 : r0 + HS, cc, tq * P : (tq + 1) * P]),
                        rhs=mm_cast(khT[r0 : r0 + HS, cc, :]),
                        start=True,
                        stop=True,
                    )
                    u = u_all[:, tq, h, :]
                    dsum = small.tile([P, 1], f32, tag="dsum")
                    nc.scalar.activation(
                        u,
                        s_ps,
                        mybir.ActivationFunctionType.Exp,
                        scale=SCALE,
                        accum_out=dsum,
                    )
                    rc = small.tile([P, 1], f32, tag="recip", name=f"rc_{tq}_{h}")
                    nc.vector.reciprocal(rc, dsum)
                    recips[tq, h] = rc
                    nc.vector.tensor_scalar_mul(attn_f[:, h, :], u, rc)
                nc.sync.dma_start(out_attn[tq * P : (tq + 1) * P, :, :], attn_f[:])

            # ---- transpose unnormalized U: [qtok, ktok] -> [ktok, qtok]
            for h in range(GH):
                for tk in range(TK):
                    t_ps = psB.tile([P, TQ * P], dt_store, tag="t_ps")
                    for tq in range(TQ):
                        nc.tensor.transpose(
                            mm_cast(t_ps[:, tq * P : (tq + 1) * P]),
                            mm_cast(u_all[:, tq, h, tk * P : (tk + 1) * P]),
                            mm_cast(ident[:, :]),
                        )
                    nc.any.tensor_copy(ut_all[:, h, tk, :], t_ps)

            # ---- x = attn @ vh (unnormalized, then scaled by recip)
            for tq in range(TQ):
                x_ps = psC.tile([P, GC], f32, tag="x_ps")
                for h in range(GH):
                    for tk in range(TK):
                        nc.tensor.matmul(
                            x_ps[:, h * HS : (h + 1) * HS],
                            lhsT=mm_cast(ut_all[:, h, tk, tq * P : (tq + 1) * P]),
                            rhs=mm_cast(vh[:, tk, h * HS : (h + 1) * HS]),
                            start=(tk == 0),
                            stop=(tk == TK - 1),
                        )
                x_sb = work.tile([P, GC], f32, tag="x_sb")
                for h in range(GH):
                    nc.vector.tensor_scalar_mul(
                        x_sb[:, h * HS : (h + 1) * HS],
                        x_ps[:, h * HS : (h + 1) * HS],
                        recips[tq, h],
                    )
                nc.sync.dma_start(out_x[tq * P : (tq + 1) * P, :], x_sb[:])

    nc.compile()
    return nc


def _get_nc(dt_mm_name: str):
    if dt_mm_name not in _CACHE:
        _CACHE[dt_mm_name] = _build_nc(dt_mm_name)
    return _CACHE[dt_mm_name]


def _pack(a, dtype):
    # [512, F] -> [128, 4, F] partition-major contiguous
    F = a.shape[1]
    return np.ascontiguousarray(
        a.reshape(KC, P, F).transpose(1, 0, 2).astype(dtype)
    )


def make_in_maps(q, k, v, W_q, W_k, W_v, dt_mm_name=None):
    """Shard full inputs into 8 per-core input dicts (host-side transposes)."""
    dt = _np_in_dtype(dt_mm_name or DT_MM)
    in_maps = []
    packed_w = {}
    for hg in range(HG):
        sl = slice(hg * GC, (hg + 1) * GC)
        packed_w[hg] = (
            _pack(np.ascontiguousarray(W_q[sl, :].T), dt),
            _pack(np.ascontiguousarray(W_k[sl, :].T), dt),
            _pack(np.ascontiguousarray(W_v[sl, :].T), dt),
        )
    packed_x = {}
    for b in range(B):
        packed_x[b] = (
            _pack(np.ascontiguousarray(q[b].T), dt),
            _pack(np.ascontiguousarray(k[b].T), dt),
            _pack(np.ascontiguousarray(v[b].T), dt),
        )
    for c in range(2 * B):
        b, hg = c // HG, c % HG
        qTp, kTp, vTp = packed_x[b]
        wqTp, wkTp, wvTp = packed_w[hg]
        in_maps.append(
            {"qT": qTp, "kT": kTp, "vT": vTp, "wqT": wqTp, "wkT": wkTp, "wvT": wvTp}
        )
    return in_maps


def assemble(results):
    """Gather 8 per-core outputs into full (x, attn)."""
    x = np.empty((B, N, C), dtype=np.float32)
    attn = np.empty((B, H, N, N), dtype=np.float32)
    for c in range(2 * B):
        b, hg = c // HG, c % HG
        # out_attn [N, GH, N] -> [GH, N, N]
        attn[b, hg * GH : (hg + 1) * GH] = np.asarray(
            results[c]["out_attn"], dtype=np.float32
        ).transpose(1, 0, 2)
        x[b, :, hg * GC : (hg + 1) * GC] = results[c]["out_x"]
    return x, attn


def kernel(q, k, v, relation_feature=None, W_q=None, W_k=None, W_v=None,
           W_r_conv=None, W_r_qk=None, _trace=False):
    from concourse.bass_utils import run_bass_kernel_spmd

    nc = _get_nc(DT_MM)
    in_maps = make_in_maps(
        np.asarray(q), np.asarray(k), np.asarray(v),
        np.asarray(W_q), np.asarray(W_k), np.asarray(W_v),
    )
    res = run_bass_kernel_spmd(nc, in_maps, core_ids=list(range(2 * B)), trace=_trace)
    x, attn = assemble(res.results)
    if _trace:
        return (x, attn), res
    return (x, attn)
